# revision 1
# baseline (speedup 1.0000x reference)
"""Distributed embedding lookup (DistEmb forward) on 8 TRN2 NeuronCores.

Reference: out[i] = table[idx[i]] for table [2M, 128] f32, idx [1M] ints.

Sharding strategy (per the module's part_book partition scheme):
- The 1 GiB table fits HBM easily, so every core keeps a full replica
  (the limit case of the hint's "replicated hot-row cache") and the 1M
  ids are sharded contiguously 8 ways — perfectly balanced, no
  cross-core collectives needed.
- Within a core, ids are routed (host-side, at input-sharding time) to
  their owning 31250-row table partition: 64 chunks, so each local id
  fits int16 — the fast-path requirement of the InstDMAGatherAnt
  descriptor generator (Q7 CounterMachine: ~0.34 ns/row vs ~1 us per
  128 rows for the generic indirect-DMA path).
- Device per chunk: 3 x dma_gather of 768 ids (the Q7 gather kernel
  caps near 1024 indices/instruction) into SBUF, then one dense
  contiguous 1.1 MB writeback. 3-engine software pipeline with manual
  rotating semaphores: scalar=HWDGE idx loads, gpsimd=SWDGE gathers,
  sync=HWDGE writebacks; BUFS-deep double buffering.
- Host inverse-permutes the bucketed device output into final id order
  (the unshard step).

Per-core HW traffic: ~72 MB random 512B-row reads + ~72 MB contiguous
writes ~= 1.2x the 373 us HBM roofline for this op.
"""
import numpy as np

import concourse.bacc as bacc
import concourse.bass as bass
import concourse.mybir as mybir
from concourse.bass_utils import run_bass_kernel_spmd
from concourse.library_config import mlp

NUM_NODES = 2_000_000
D = 128
NUM_IDS = 1_048_576
N_CORES = 8
NPC = NUM_IDS // N_CORES      # 131072 ids per core
NCHUNK = 64
CHUNK = NUM_NODES // NCHUNK   # 31250 rows per chunk (int16-addressable)
CPAD = 2304                   # padded ids per chunk (max observed 2176)
S = CPAD // 128               # 18 free slots per partition in gather tile
NSUB = 3                      # sub-gathers per chunk (Q7 caps ~1024 idx/inst)
L = CPAD // NSUB              # 768 ids per sub-gather
BUFS = 3

_prog_cache = {}


def build_program():
    key = ("v2", CPAD, BUFS, NSUB)
    if key in _prog_cache:
        return _prog_cache[key]
    nc = bacc.Bacc("TRN2", target_bir_lowering=False, debug=False)
    table = nc.dram_tensor(
        "table", [NUM_NODES, D], mybir.dt.float32, kind="ExternalInput"
    )
    idx16 = nc.dram_tensor(
        "idx16", [NCHUNK, 128, CPAD // 16], mybir.dt.int16, kind="ExternalInput"
    )
    out = nc.dram_tensor(
        "out", [NCHUNK * CPAD, D], mybir.dt.float32, kind="ExternalOutput"
    )
    table_chunks = table[:].rearrange("(c r) d -> c r d", r=CHUNK)

    with (
        nc.Block() as block,
        nc.semaphore("isem") as isem,
        nc.semaphore("gsem") as gsem,
        nc.semaphore("wsem") as wsem,
    ):
        idx_bufs = [
            nc.alloc_sbuf_tensor(f"idxs{b}", [128, CPAD // 16], mybir.dt.int16)
            for b in range(BUFS)
        ]
        gat_bufs = [
            nc.alloc_sbuf_tensor(f"gat{b}", [128, S, D], mybir.dt.float32)
            for b in range(BUFS)
        ]

        @block.scalar
        def _(scalar: bass.BassEngine):
            for c in range(NCHUNK):
                if c >= BUFS:
                    # WAR: idx buf free once gathers of chunk c-BUFS consumed it
                    scalar.wait_ge(gsem, 16 * NSUB * (c - BUFS + 1))
                scalar.dma_start(
                    idx_bufs[c % BUFS][:], idx16[c, :, :]
                ).then_inc(isem, 16)

        @block.gpsimd
        def _(gpsimd: bass.BassGpSimd):
            gpsimd.load_library(mlp)
            for c in range(NCHUNK):
                gpsimd.wait_ge(isem, 16 * (c + 1))
                if c >= BUFS:
                    # WAR: gather buf free once writeback c-BUFS done
                    gpsimd.wait_ge(wsem, 16 * (c - BUFS + 1))
                gat = gat_bufs[c % BUFS]
                idxs = idx_bufs[c % BUFS]
                for g in range(NSUB):
                    gpsimd.dma_gather(
                        gat[:, g * (L // 128):(g + 1) * (L // 128), :],
                        table_chunks[c],
                        idxs[:, g * (L // 16):(g + 1) * (L // 16)],
                        L,
                        L,
                        D,
                    ).then_inc(gsem, 16)

        @block.sync
        def _(sync: bass.BassEngine):
            for c in range(NCHUNK):
                sync.wait_ge(gsem, 16 * NSUB * (c + 1))
                sync.dma_start(
                    out[c * CPAD:(c + 1) * CPAD, :].rearrange(
                        "(p s) d -> p (s d)", p=128
                    ),
                    gat_bufs[c % BUFS][:].rearrange("p s d -> p (s d)"),
                ).then_inc(wsem, 16)
            sync.wait_ge(wsem, 16 * NCHUNK)

    nc.compile()
    _prog_cache[key] = nc
    return nc


def _route_core(ids32):
    """Bucket one core's ids by owning table chunk.

    Returns (idx16 [NCHUNK,128,CPAD//16] wrapped+padded local ids,
    src_rows [NPC] device-output row of each bucket-ordered id,
    order [NPC] argsort positions)."""
    chunk_of = ids32 // CHUNK
    order = np.argsort(chunk_of, kind="stable")
    sorted_ids = ids32[order]
    sorted_chunks = chunk_of[order]
    counts = np.bincount(sorted_chunks, minlength=NCHUNK)
    if counts.max() > CPAD:
        raise ValueError(f"chunk bucket overflow: {counts.max()} > {CPAD}")
    local = (sorted_ids - sorted_chunks * CHUNK).astype(np.int16)

    idx16 = np.empty((NCHUNK, CPAD), dtype=np.int16)
    starts = np.zeros(NCHUNK + 1, dtype=np.int64)
    np.cumsum(counts, out=starts[1:])
    j_within = np.arange(len(ids32), dtype=np.int64) - starts[sorted_chunks]
    for c in range(NCHUNK):
        n = counts[c]
        seg = local[starts[c]:starts[c + 1]]
        idx16[c, :n] = seg
        # pad with a duplicate valid id (static num_idxs, no dynamic counts)
        idx16[c, n:] = seg[0] if n else 0
    # device row of bucket-ordered element j: sub-gather g = j//L writes
    # local j%L to partition (j%L)%128, slot g*(L//128) + (j%L)//128; the
    # contiguous writeback puts SBUF (p, s) at DRAM row c*CPAD + p*S + s.
    g_sub = j_within // L
    j_local = j_within % L
    src_rows = (
        sorted_chunks.astype(np.int64) * CPAD
        + (j_local % 128) * S
        + g_sub * (L // 128)
        + j_local // 128
    )
    # wrap for the Q7 index reader: id j at partition j%16, column j//16
    # (identical per-sub-gather and globally since L%16==0), replicated
    # across the 8 groups of 16 partitions.
    wrapped = idx16.reshape(NCHUNK, CPAD // 16, 16).transpose(0, 2, 1)
    rep = np.broadcast_to(
        wrapped[:, None, :, :], (NCHUNK, 8, 16, CPAD // 16)
    ).reshape(NCHUNK, 128, CPAD // 16)
    return np.ascontiguousarray(rep), src_rows, order


def kernel(table, idx):
    table = np.ascontiguousarray(np.asarray(table), dtype=np.float32)
    idx32 = np.ascontiguousarray(np.asarray(idx)).astype(np.int32)
    nc = build_program()

    in_maps = []
    routing = []
    for c in range(N_CORES):
        ids = idx32[c * NPC:(c + 1) * NPC]
        idx16, src_rows, order = _route_core(ids)
        in_maps.append({"table": table, "idx16": idx16})
        routing.append((src_rows, order))

    res = run_bass_kernel_spmd(nc, in_maps, core_ids=list(range(N_CORES)))

    out = np.empty((NUM_IDS, D), dtype=np.float32)
    for c in range(N_CORES):
        src_rows, order = routing[c]
        dev = res.results[c]["out"]
        blk = out[c * NPC:(c + 1) * NPC]
        blk[order] = dev[src_rows]
    return out



# revision 6
# speedup vs baseline: 1.0308x; 1.0308x over previous
"""Distributed embedding lookup (DistEmb forward) on 8 TRN2 NeuronCores.

Reference: out[i] = table[idx[i]] for table [2M, 128] f32, idx [1M] ints.

v2 strategy (row-wise table shard per the module's part_book scheme):
- The table is sharded row-wise: core c binds ONLY its 250k-row window
  as the "table" input (same SPMD program; different per-core binding).
  The 1M ids are globally sorted and routed host-side to the owning
  core (host-side all-to-all — the unshard step is host-side anyway),
  so each core's reads are a MONOTONIC ~2-row-stride stream inside a
  128 MB window: near-sequential HBM access instead of the 15-row
  random strides a replicated table gives. Upload is 1 GB, not 8 GB.
- Within a core: 8 chunks of 31250 rows (int16-addressable for the
  InstDMAGatherAnt fast path), CPAD=18432 slots per chunk. Device
  pipeline identical to the proven baseline: per 2304-row tile, 3 x
  dma_gather of 768 sorted ids into an SBUF tile, then one dense
  1.18 MB writeback; 64 tiles per core.
- Chunk buckets are padded with a duplicate of their last valid id
  (the Q7 desc count must match the decode-side ring reservation, so
  negative-id trimming is not safe); dup reads are same-row HBM
  row-buffer hits, near-free.
- idx16 is preloaded in ONE contiguous 2.25 MB DMA (vs 64 per-chunk
  loads), freeing the scalar engine and an SBUF rotation.
- 3-engine pipeline with counters: scalar=idx preload, gpsimd=SWDGE
  gathers (queue 0, in-order), sync=HWDGE writebacks; BUFS-deep
  rotation.
- Host inverse-permutes the bucket-ordered device outputs into final
  id order.
"""
import numpy as np

import concourse.bacc as bacc
import concourse.bass as bass
import concourse.mybir as mybir
from concourse.bass_utils import run_bass_kernel_spmd
from concourse.library_config import mlp

NUM_NODES = 2_000_000
D = 128
NUM_IDS = 1_048_576
N_CORES = 8
WIN = NUM_NODES // N_CORES    # 250000 rows per core window
NCHUNK = 8                    # int16-addressable chunks per window
CHUNK = WIN // NCHUNK         # 31250
CPAD = 18432                  # padded slots per chunk (mean 16384, +16 sigma)
TILE = 2304                   # rows per SBUF tile / writeback
NSUB = 3                      # gathers per tile
L = TILE // NSUB              # 768 ids per dma_gather
TPC = CPAD // TILE            # 8 tiles per chunk
NTILE = NCHUNK * TPC          # 64 tiles per core
ST = TILE // 128              # 18 slots per partition
W = CPAD // 16                # idx columns per chunk
BUFS = 4

_prog_cache = {}


def build_program():
    key = ("v2c", CPAD, TILE, BUFS)
    if key in _prog_cache:
        return _prog_cache[key]
    nc = bacc.Bacc("TRN2", target_bir_lowering=False, debug=False)
    table = nc.dram_tensor("table", [WIN, D], mybir.dt.float32,
                           kind="ExternalInput")
    idx16 = nc.dram_tensor("idx16", [128, NCHUNK * W], mybir.dt.int16,
                           kind="ExternalInput")
    out = nc.dram_tensor("out", [NCHUNK * CPAD, D], mybir.dt.float32,
                         kind="ExternalOutput")
    table_chunks = table[:].rearrange("(c r) d -> c r d", r=CHUNK)

    with (nc.Block() as block, nc.semaphore("isem") as isem,
          nc.semaphore("gsem") as gsem, nc.semaphore("wsem") as wsem):
        idx_all = nc.alloc_sbuf_tensor("idx_all", [128, NCHUNK * W],
                                       mybir.dt.int16)
        gat_bufs = [nc.alloc_sbuf_tensor(f"gat{b}", [128, ST, D],
                                         mybir.dt.float32)
                    for b in range(BUFS)]

        @block.scalar
        def _(scalar: bass.BassEngine):
            scalar.dma_start(idx_all[:], idx16[:]).then_inc(isem, 16)

        @block.gpsimd
        def _(gpsimd: bass.BassGpSimd):
            gpsimd.load_library(mlp)
            gpsimd.wait_ge(isem, 16)
            for t in range(NTILE):
                c, tl = divmod(t, TPC)
                if t >= BUFS:
                    # WAR: tile buf free once writeback t-BUFS done
                    gpsimd.wait_ge(wsem, 16 * (t - BUFS + 1))
                for g in range(NSUB):
                    col = c * W + (tl * TILE + g * L) // 16
                    gpsimd.dma_gather(
                        gat_bufs[t % BUFS][:, g * (L // 128):
                                           (g + 1) * (L // 128), :],
                        table_chunks[c],
                        idx_all[:, col:col + L // 16],
                        L, L, D,
                    ).then_inc(gsem, 16)

        @block.sync
        def _(sync: bass.BassEngine):
            for t in range(NTILE):
                sync.wait_ge(gsem, 16 * NSUB * (t + 1))
                sync.dma_start(
                    out[t * TILE:(t + 1) * TILE, :].rearrange(
                        "(p s) d -> p (s d)", p=128),
                    gat_bufs[t % BUFS][:].rearrange("p s d -> p (s d)"),
                ).then_inc(wsem, 16)
            sync.wait_ge(wsem, 16 * NTILE)

    nc.compile()
    _prog_cache[key] = nc
    return nc


def prepare(table, idx):
    """Host routing: global sort, shard by 250k-row window, bucket by
    31250-row chunk, pad with a duplicate valid id (static num_idxs).

    Returns (in_maps, (order, core_of, src_rows)) where
    out[order[k]] = results[core_of[k]]["out"][src_rows[k]]."""
    table = np.ascontiguousarray(np.asarray(table), dtype=np.float32)
    idx32 = np.ascontiguousarray(np.asarray(idx)).astype(np.int32)

    order = np.argsort(idx32, kind="stable")
    sorted_ids = idx32[order]
    chunk64 = sorted_ids // CHUNK            # global chunk 0..63
    core_of = chunk64 // NCHUNK
    counts = np.bincount(chunk64, minlength=N_CORES * NCHUNK)
    if counts.max() > CPAD:
        raise ValueError(f"chunk bucket overflow: {counts.max()} > {CPAD}")
    starts = np.zeros(N_CORES * NCHUNK + 1, dtype=np.int64)
    np.cumsum(counts, out=starts[1:])
    j_within = np.arange(len(sorted_ids), dtype=np.int64) - starts[chunk64]
    local = (sorted_ids - chunk64 * CHUNK).astype(np.int16)

    # device output row (within the owning core's out tensor): tile
    # tl=j//TILE of chunk cc; within tile, sub-gather g2=(j%TILE)//L
    # writes id il=j%L to partition il%128, slot g2*(L//128)+il//128;
    # writeback t=cc*TPC+tl lands SBUF (p,s) at row t*TILE + p*ST + s.
    cc_local = chunk64 - core_of * NCHUNK
    tl = j_within // TILE
    i = j_within % TILE
    g2 = i // L
    il = i % L
    src_rows = (cc_local * CPAD + tl * TILE
                + (il % 128) * ST + g2 * (L // 128) + il // 128)

    in_maps = []
    for c in range(N_CORES):
        idx16 = np.zeros((NCHUNK, CPAD), dtype=np.int16)
        for k in range(NCHUNK):
            cc = c * NCHUNK + k
            n = counts[cc]
            seg = local[starts[cc]:starts[cc + 1]]
            idx16[k, :n] = seg
            idx16[k, n:] = seg[n - 1] if n else 0
        wrapped = idx16.reshape(NCHUNK, W, 16).transpose(0, 2, 1)
        rep = np.broadcast_to(wrapped[:, None, :, :], (NCHUNK, 8, 16, W))
        rep = rep.reshape(NCHUNK, 128, W).transpose(1, 0, 2)
        in_maps.append({
            "table": table[c * WIN:(c + 1) * WIN],
            "idx16": np.ascontiguousarray(rep.reshape(128, NCHUNK * W)),
        })
    return in_maps, (order, core_of, src_rows)


def kernel(table, idx):
    nc = build_program()
    in_maps, (order, core_of, src_rows) = prepare(table, idx)
    res = run_bass_kernel_spmd(nc, in_maps, core_ids=list(range(N_CORES)))

    out = np.empty((NUM_IDS, D), dtype=np.float32)
    for c in range(N_CORES):
        sel = core_of == c
        out[order[sel]] = res.results[c]["out"][src_rows[sel]]
    return out
